# revision 1
# baseline (speedup 1.0000x reference)
"""GNN message-passing kernel for 8 Trainium2 NeuronCores.

Math: 3 layers of
    m   = relu(relu(x[src] @ Wm1 + bm1) @ Wm2 + bm2)      # message MLP
    aggr= segment_sum(m, dst)                              # scatter-add
    x   = relu(relu([aggr, x] @ Wu1 + bu1) @ Wu2 + bu2)    # update MLP

Key algorithmic point: the message MLP depends only on x[src], so it is
computed per-NODE (100k rows) not per-EDGE (1.6M rows); the edge work
reduces to gather + segment-sum, done as:
  - per-layer AllGather of the per-node message table (bf16, padded rows)
  - dma_gather of 256B rows (4 passes over 25088-row chunks: int16 index)
  - segment-sum via PE matmul against a one-hot slot matrix built on DVE
"""

import sys

sys.path.insert(0, "/opt/trn_rl_repo")

import os
import numpy as np
import ml_dtypes

KPHASE = int(os.environ.get("KPHASE", "5"))
KLAYERS = int(os.environ.get("KLAYERS", "3"))

N_NODES = 100000
N_EDGES = 1600000
D = 64
H = 16
N_LAYERS = 3
NCORES = 8

B = 12500          # nodes per core
BP = 12544         # padded (98 * 128)
NSW = 98           # subwindows of 128 nodes per core
CH = 2 * BP        # 25088 rows per gather chunk (2 blocks)
NCHUNK = 4
PADIDX = B         # chunk-relative index of a guaranteed-zero table row
NWIN = 25          # windows of 512 nodes (last = 256)
SUB_TILES = 64     # gather sub-call size in 128-edge tiles (8192 idxs)

_BF16 = ml_dtypes.bfloat16


def _preprocess(edge_index):
    """Build per-core padded gather-index and slot arrays (layer-invariant)."""
    src = np.asarray(edge_index[0], dtype=np.int64)
    dst = np.asarray(edge_index[1], dtype=np.int64)
    core = dst // B
    dstL = dst - core * B
    sw = dstL >> 7
    slot = (dstL & 127).astype(np.uint8)
    src_row = (src // B) * BP + (src % B)
    chunk = src_row // CH
    rel = (src_row - chunk * CH).astype(np.int16)

    key = ((core * NCHUNK + chunk) * NSW + sw).astype(np.int64)
    order = np.argsort(key, kind="stable")
    key_s = key[order]
    rel_s = rel[order]
    slot_s = slot[order]
    core_s = core[order]
    chunk_s = chunk[order]

    counts = np.bincount(key, minlength=NCORES * NCHUNK * NSW).reshape(
        NCORES, NCHUNK, NSW
    )
    # tiles per (chunk, sw): max over cores, >= 1
    T = np.maximum(1, -(-counts // 128)).max(axis=0)  # [NCHUNK, NSW]
    cap = T * 128

    grp_start = np.zeros(NCORES * NCHUNK * NSW, dtype=np.int64)
    grp_start[1:] = np.cumsum(counts.ravel())[:-1]
    within = np.arange(len(key_s), dtype=np.int64) - grp_start[key_s]

    base = np.zeros((NCHUNK, NSW), dtype=np.int64)
    base[:, 1:] = np.cumsum(cap, axis=1)[:, :-1]
    dest = base[chunk_s, key_s % NSW] + within

    Lk = cap.sum(axis=1).astype(np.int64)  # padded edge-slots per chunk
    idx_pads = []
    slot_pads = []
    for k in range(NCHUNK):
        ip = np.full((NCORES, Lk[k]), PADIDX, dtype=np.int16)
        sp = np.zeros((NCORES, Lk[k]), dtype=np.uint8)
        m = chunk_s == k
        flat = core_s[m] * Lk[k] + dest[m]
        ip.reshape(-1)[flat] = rel_s[m]
        sp.reshape(-1)[flat] = slot_s[m]
        idx_pads.append(ip)
        slot_pads.append(sp)

    # wrap: idx position i -> [i%16, i//16]; replicate rows 0-15 into 16-31
    idx_wrapped = []
    slot_wrapped = []
    for k in range(NCHUNK):
        iw = np.zeros((NCORES, 128, Lk[k] // 16), dtype=np.int16)
        w = idx_pads[k].reshape(NCORES, -1, 16).transpose(0, 2, 1)
        iw[:, 0:16] = w
        iw[:, 16:32] = w
        idx_wrapped.append(iw)
        slot_wrapped.append(
            slot_pads[k]
            .reshape(NCORES, -1, 128)
            .transpose(0, 2, 1)
            .astype(_BF16)
        )
    idx_all = np.concatenate(idx_wrapped, axis=2)    # [8, 128, sum(Lk)/16]
    slot_all = np.concatenate(slot_wrapped, axis=2)  # [8, 128, sum(Lk)/128]
    return T, Lk, idx_all, slot_all


def _build_program(T, Lk):
    """Build the SPMD bass program (identical for all 8 cores)."""
    import concourse.bass as bass
    import concourse.bacc as bacc
    import concourse.tile as tile
    from concourse import mybir

    dt = mybir.dt
    AF = mybir.ActivationFunctionType
    ALU = mybir.AluOpType

    nc = bacc.Bacc(
        "TRN2", target_bir_lowering=False, debug=False, num_devices=NCORES
    )

    # ---- I/O ----
    x0_d = nc.dram_tensor("x0", [BP, D], dt.float32, kind="ExternalInput")
    idx_cols = int(Lk.sum()) // 16
    slot_cols = int(Lk.sum()) // 128
    idx_d = nc.dram_tensor("idx", [128, idx_cols], dt.int16, kind="ExternalInput")
    slot_d = nc.dram_tensor(
        "slot", [128, slot_cols], dt.bfloat16, kind="ExternalInput"
    )
    wm1_d = nc.dram_tensor("wm1", [N_LAYERS, D, H], dt.bfloat16, kind="ExternalInput")
    bm1_d = nc.dram_tensor("bm1", [N_LAYERS, H, 1], dt.float32, kind="ExternalInput")
    wm2_d = nc.dram_tensor("wm2", [N_LAYERS, H, D], dt.bfloat16, kind="ExternalInput")
    bm2_d = nc.dram_tensor("bm2", [N_LAYERS, D, 1], dt.float32, kind="ExternalInput")
    wu1a_d = nc.dram_tensor("wu1a", [N_LAYERS, D, H], dt.float32, kind="ExternalInput")
    wu1b_d = nc.dram_tensor(
        "wu1b", [N_LAYERS, D, H], dt.bfloat16, kind="ExternalInput"
    )
    bu1_d = nc.dram_tensor("bu1", [N_LAYERS, H, 1], dt.float32, kind="ExternalInput")
    wu2_d = nc.dram_tensor("wu2", [N_LAYERS, H, D], dt.bfloat16, kind="ExternalInput")
    bu2_d = nc.dram_tensor("bu2", [N_LAYERS, D, 1], dt.float32, kind="ExternalInput")
    iota_d = nc.dram_tensor("iota", [128, 128], dt.bfloat16, kind="ExternalInput")
    idenf_d = nc.dram_tensor("idenf", [128, 128], dt.float32, kind="ExternalInput")
    idenb_d = nc.dram_tensor("idenb", [64, 64], dt.bfloat16, kind="ExternalInput")
    out_d = nc.dram_tensor("out", [BP, D], dt.float32, kind="ExternalOutput")

    block_d = nc.dram_tensor("mblock", [BP, 128], dt.bfloat16)
    table_d = nc.dram_tensor(
        "mtable", [NCORES * BP, 128], dt.bfloat16, addr_space="Shared"
    )

    # per-chunk tile schedule
    sw_of_tile = []  # [NCHUNK][tile] -> sw
    tflag = []       # [NCHUNK][tile] -> (is_first, is_last) within (chunk, sw)
    for k in range(NCHUNK):
        sws = []
        fl = []
        for sw in range(NSW):
            t = int(T[k][sw])
            for j in range(t):
                sws.append(sw)
                fl.append((j == 0, j == t - 1))
        sw_of_tile.append(sws)
        tflag.append(fl)
    tiles_k = [len(s) for s in sw_of_tile]
    idx_off16 = np.concatenate([[0], np.cumsum(Lk // 16)]).astype(int)
    slot_offT = np.concatenate([[0], np.cumsum(Lk // 128)]).astype(int)

    from contextlib import ExitStack

    with tile.TileContext(nc) as tc, ExitStack() as ctx:
        P = lambda **kw: ctx.enter_context(tc.tile_pool(**kw))
        cpool = P(name="consts", bufs=1)
        xpool = P(name="xT", bufs=1)
        apool = P(name="aggr", bufs=1)
        slpool = P(name="slot", bufs=1)
        idxpool = P(name="idx", bufs=2)
        gpool = P(name="G", bufs=2)
        spool = P(name="S", bufs=2)
        mpool = P(name="mT", bufs=2)
        hpool = P(name="hid", bufs=2)
        stgpool = P(name="stg", bufs=3)
        xldpool = P(name="xld", bufs=2)
        xopool = P(name="xout", bufs=2)
        ps_seg = P(name="ps_seg", bufs=2, space="PSUM")
        ps_m16 = P(name="ps_m16", bufs=2, space="PSUM")
        ps_m64 = P(name="ps_m64", bufs=2, space="PSUM")
        ps_tr = P(name="ps_tr", bufs=2, space="PSUM")

        # ---- constants ----
        iota_t = cpool.tile([128, 128], dt.bfloat16, tag="iota")
        nc.sync.dma_start(iota_t[:], iota_d[:])
        idenf_t = cpool.tile([128, 128], dt.float32, tag="idenf")
        nc.sync.dma_start(idenf_t[:], idenf_d[:])
        idenb_t = cpool.tile([64, 64], dt.bfloat16, tag="idenb")
        nc.sync.dma_start(idenb_t[:], idenb_d[:])

        def _w(name, dram, shape, dtype):
            ts = []
            for l in range(N_LAYERS):
                t = cpool.tile(shape, dtype, tag=f"{name}{l}")
                nc.sync.dma_start(t[:], dram[l])
                ts.append(t)
            return ts

        wm1_t = _w("wm1", wm1_d, [D, H], dt.bfloat16)
        bm1_t = _w("bm1", bm1_d, [H, 1], dt.float32)
        wm2_t = _w("wm2", wm2_d, [H, D], dt.bfloat16)
        bm2_t = _w("bm2", bm2_d, [D, 1], dt.float32)
        wu1a_t = _w("wu1a", wu1a_d, [D, H], dt.float32)
        wu1b_t = _w("wu1b", wu1b_d, [D, H], dt.bfloat16)
        bu1_t = _w("bu1", bu1_d, [H, 1], dt.float32)
        wu2_t = _w("wu2", wu2_d, [H, D], dt.bfloat16)
        bu2_t = _w("bu2", bu2_d, [D, 1], dt.float32)

        slot_t = slpool.tile([128, slot_cols], dt.bfloat16)
        nc.sync.dma_start(slot_t[:], slot_d[:])

        # ---- x0 -> x^T (bf16) ----
        xA = xpool.tile([D, BP], dt.bfloat16, tag="xA")
        nc.gpsimd.memset(xA[:, B:BP], 0.0)
        for t in range(NSW):
            xld = xldpool.tile([128, D], dt.float32)
            nc.sync.dma_start(xld[:], x0_d[128 * t : 128 * (t + 1), :])
            pt = ps_tr.tile([D, 128], dt.float32)
            nc.tensor.transpose(pt[:], xld[:], idenf_t[:])
            nc.scalar.activation(xA[:, 128 * t : 128 * (t + 1)], pt[:], AF.Copy)

        aggr = apool.tile([D, BP], dt.float32)

        for layer in range(KLAYERS):
            xT = xA
            xN = xA  # in-place update: xT[ch] is dead once the update MLP read it

            # ---- message MLP + transpose + block write ----
            for ch in range(NWIN):
                o = 512 * ch
                wd = min(512, B - o)         # mlp width (real nodes)
                wt = min(512, BP - o)        # transpose width (padded)
                p1 = ps_m16.tile([H, 512], dt.float32)
                nc.tensor.matmul(
                    p1[:, :wd], lhsT=wm1_t[layer][:], rhs=xT[:, o : o + wd],
                    start=True, stop=True,
                )
                hid = hpool.tile([H, 512], dt.bfloat16, tag="hid")
                nc.scalar.activation(
                    hid[:, :wd], p1[:, :wd], AF.Relu, bias=bm1_t[layer][:]
                )
                p2 = ps_m64.tile([D, 512], dt.float32)
                nc.tensor.matmul(
                    p2[:, :wd], lhsT=wm2_t[layer][:], rhs=hid[:, :wd],
                    start=True, stop=True,
                )
                mt = mpool.tile([D, 512], dt.bfloat16)
                nc.scalar.activation(
                    mt[:, :wd], p2[:, :wd], AF.Relu, bias=bm2_t[layer][:]
                )
                if wd < wt:
                    nc.gpsimd.memset(mt[:, wd:wt], 0.0)
                for st in range(-(-wt // 128)):
                    so = 128 * st
                    pt = ps_tr.tile([128, D], dt.bfloat16)
                    nc.tensor.transpose(
                        pt[:], mt[:, so : so + 128], idenb_t[:]
                    )
                    stg = stgpool.tile([128, D], dt.bfloat16, tag="stgb")
                    nc.scalar.activation(stg[:], pt[:], AF.Copy)
                    nc.sync.dma_start(
                        block_d[o + so : o + so + 128, 0:D], stg[:]
                    )

            # ---- AllGather message table ----
            if KPHASE >= 2:
                nc.gpsimd.collective_compute(
                    "AllGather",
                    mybir.AluOpType.bypass,
                    replica_groups=[list(range(NCORES))],
                    ins=[block_d[:].opt()],
                    outs=[table_d[:].opt()],
                )

            # ---- gather + segment matmul passes ----
            for k in range(NCHUNK if KPHASE >= 3 else 0):
                ncols = int(Lk[k]) // 16
                idx_t = idxpool.tile([128, ncols], dt.int16)
                nc.sync.dma_start(
                    idx_t[:], idx_d[:, idx_off16[k] : idx_off16[k] + ncols]
                )
                table_k = table_d[CH * k : CH * (k + 1), :]

                ntiles = tiles_k[k]
                nsub = -(-ntiles // SUB_TILES)
                cur_win = -1
                pw = None
                tb = 0  # tile base within this chunk
                for si in range(nsub):
                    nt = min(SUB_TILES, ntiles - tb)
                    g = gpool.tile([128, SUB_TILES, 128], dt.bfloat16)
                    nidx = nt * 128
                    nc.gpsimd.dma_gather(
                        g[:, :nt, :],
                        table_k,
                        idx_t[:, 8 * tb : 8 * (tb + nt)],
                        nidx,
                        nidx,
                        128,
                        single_packet=False,
                    )
                    sb = spool.tile([128, SUB_TILES, 128], dt.bfloat16)
                    io_b = iota_t[:].unsqueeze(1).broadcast_to([128, nt, 128])
                    sl_b = (
                        slot_t[:, slot_offT[k] + tb : slot_offT[k] + tb + nt]
                        .unsqueeze(2)
                        .broadcast_to([128, nt, 128])
                    )
                    nc.vector.tensor_tensor(sb[:, :nt, :], io_b, sl_b, ALU.is_equal)

                    for j in range(nt if KPHASE >= 4 else 0):
                        ti = tb + j
                        sw = sw_of_tile[k][ti]
                        first, last = tflag[k][ti]
                        win = sw // 4
                        if win != cur_win:
                            if pw is not None:
                                _evac(nc, ALU, aggr, pw, cur_win, k)
                            pw = ps_seg.tile([D, 512], dt.float32)
                            cur_win = win
                        nc.tensor.matmul(
                            pw[:, 128 * (sw % 4) : 128 * (sw % 4) + 128],
                            lhsT=g[:, j, 0:D],
                            rhs=sb[:, j, :],
                            start=first,
                            stop=last,
                        )
                    tb += nt
                if pw is not None:
                    _evac(nc, ALU, aggr, pw, cur_win, k)

            # ---- update MLP ----
            if KPHASE < 4:
                nc.vector.tensor_copy(aggr[:], xT[:])  # placeholder so aggr defined
            for ch in range(NWIN if KPHASE >= 5 else 0):
                o = 512 * ch
                wd = min(512, B - o)
                p1 = ps_m16.tile([H, 512], dt.float32)
                nc.tensor.matmul(
                    p1[:, :wd], lhsT=wu1a_t[layer][:], rhs=aggr[:, o : o + wd],
                    start=True, stop=False, skip_group_check=True,
                )
                nc.tensor.matmul(
                    p1[:, :wd], lhsT=wu1b_t[layer][:], rhs=xT[:, o : o + wd],
                    start=False, stop=True, skip_group_check=True,
                )
                hid = hpool.tile([H, 512], dt.bfloat16, tag="hid")
                nc.scalar.activation(
                    hid[:, :wd], p1[:, :wd], AF.Relu, bias=bu1_t[layer][:]
                )
                p2 = ps_m64.tile([D, 512], dt.float32)
                nc.tensor.matmul(
                    p2[:, :wd], lhsT=wu2_t[layer][:], rhs=hid[:, :wd],
                    start=True, stop=True,
                )
                if layer < N_LAYERS - 1:
                    nc.scalar.activation(
                        xN[:, o : o + wd], p2[:, :wd], AF.Relu,
                        bias=bu2_t[layer][:],
                    )
                else:
                    xo = xopool.tile([D, 512], dt.float32)
                    nc.scalar.activation(
                        xo[:, :wd], p2[:, :wd], AF.Relu, bias=bu2_t[layer][:]
                    )
                    for st in range(-(-wd // 128)):
                        so = 128 * st
                        tw = min(128, wd - so)
                        pt = ps_tr.tile([128, D], dt.float32)
                        nc.tensor.transpose(
                            pt[:tw, :], xo[:, so : so + tw], idenf_t[0:D, 0:D]
                        )
                        stg = stgpool.tile([128, D], dt.float32, tag="stgf")
                        nc.scalar.activation(stg[:tw, :], pt[:tw, :], AF.Copy)
                        nc.sync.dma_start(
                            out_d[o + so : o + so + tw, :], stg[:tw, :]
                        )
        if KPHASE < 5:
            zo = stgpool.tile([128, D], dt.float32, tag="stgf")
            nc.gpsimd.memset(zo[:], 0.0)
            for ch0 in range(NSW):
                nc.sync.dma_start(out_d[128 * ch0 : 128 * (ch0 + 1), :], zo[:])
    nc.finalize()
    return nc


def _evac(nc, ALU, aggr, pw, win, k):
    o = 512 * win
    wd = min(512, BP - o)
    if k == 0:
        nc.vector.tensor_copy(aggr[:, o : o + wd], pw[:, :wd])
    else:
        nc.vector.tensor_tensor(
            aggr[:, o : o + wd], pw[:, :wd], aggr[:, o : o + wd], ALU.add
        )


_CACHE = {}


def _make_in_maps(inputs, idx_all, slot_all):
    x = np.asarray(inputs["x"], dtype=np.float32)
    Wm1, bm1 = inputs["Wm1"], inputs["bm1"]
    Wm2, bm2 = inputs["Wm2"], inputs["bm2"]
    Wu1, bu1 = inputs["Wu1"], inputs["bu1"]
    Wu2, bu2 = inputs["Wu2"], inputs["bu2"]

    xpad = np.zeros((NCORES, BP, D), dtype=np.float32)
    xpad[:, :B] = x.reshape(NCORES, B, D)

    iota = np.broadcast_to(np.arange(128, dtype=np.float32), (128, 128)).astype(_BF16)
    idenf = np.eye(128, dtype=np.float32)
    idenb = np.eye(64, dtype=np.float32).astype(_BF16)

    com = {
        "wm1": np.ascontiguousarray(np.asarray(Wm1, np.float32)).astype(_BF16),
        "bm1": np.asarray(bm1, np.float32).reshape(N_LAYERS, H, 1),
        "wm2": np.ascontiguousarray(np.asarray(Wm2, np.float32)).astype(_BF16),
        "bm2": np.asarray(bm2, np.float32).reshape(N_LAYERS, D, 1),
        "wu1a": np.ascontiguousarray(np.asarray(Wu1, np.float32)[:, :D, :]),
        "wu1b": np.ascontiguousarray(np.asarray(Wu1, np.float32)[:, D:, :]).astype(
            _BF16
        ),
        "bu1": np.asarray(bu1, np.float32).reshape(N_LAYERS, H, 1),
        "wu2": np.ascontiguousarray(np.asarray(Wu2, np.float32)).astype(_BF16),
        "bu2": np.asarray(bu2, np.float32).reshape(N_LAYERS, D, 1),
        "iota": iota,
        "idenf": idenf,
        "idenb": idenb,
    }
    in_maps = []
    for c in range(NCORES):
        m = dict(com)
        m["x0"] = xpad[c]
        m["idx"] = np.ascontiguousarray(idx_all[c])
        m["slot"] = np.ascontiguousarray(slot_all[c])
        in_maps.append(m)
    return in_maps


def kernel(x, edge_index, Wm1, bm1, Wm2, bm2, Wu1, bu1, Wu2, bu2):
    from concourse.bass_utils import run_bass_kernel_spmd

    ei_key = hash(np.asarray(edge_index)[:, ::97].tobytes())
    if ei_key not in _CACHE:
        T, Lk, idx_all, slot_all = _preprocess(edge_index)
        nc = _build_program(T, Lk)
        _CACHE[ei_key] = (nc, idx_all, slot_all)
    nc, idx_all, slot_all = _CACHE[ei_key]

    in_maps = _make_in_maps(
        dict(
            x=x, edge_index=edge_index, Wm1=Wm1, bm1=bm1, Wm2=Wm2, bm2=bm2,
            Wu1=Wu1, bu1=bu1, Wu2=Wu2, bu2=bu2,
        ),
        idx_all,
        slot_all,
    )
    res = run_bass_kernel_spmd(nc, in_maps, core_ids=list(range(NCORES)))
    out = np.concatenate([res.results[c]["out"][:B] for c in range(NCORES)], axis=0)
    return out


if __name__ == "__main__":
    import reference

    inputs = reference.setup_inputs()
    inputs = {k: np.asarray(v) for k, v in inputs.items()}
    got = kernel(**inputs)
    exp = np.asarray(reference.reference(**{k: v for k, v in inputs.items()}))
    err = np.abs(got - exp)
    rel = np.linalg.norm(got - exp) / np.linalg.norm(exp)
    print("max abs err:", err.max(), "rel:", rel)



# revision 3
# speedup vs baseline: 1.3689x; 1.3689x over previous
"""GNN message-passing kernel for 8 Trainium2 NeuronCores.

Math: 3 layers of
    m   = relu(relu(x[src] @ Wm1 + bm1) @ Wm2 + bm2)      # message MLP
    aggr= segment_sum(m, dst)                              # scatter-add
    x   = relu(relu([aggr, x] @ Wu1 + bu1) @ Wu2 + bu2)    # update MLP

Key algorithmic point: the message MLP depends only on x[src], so it is
computed per-NODE (100k rows) not per-EDGE (1.6M rows); the edge work
reduces to gather + segment-sum, done as:
  - per-layer AllGather of the per-node message table (bf16, padded rows)
  - dma_gather of 256B rows (4 passes over 25088-row chunks: int16 index)
  - segment-sum via PE matmul against a one-hot slot matrix built on DVE
"""

import sys

sys.path.insert(0, "/opt/trn_rl_repo")

import os
import numpy as np
import ml_dtypes

KPHASE = int(os.environ.get("KPHASE", "5"))
KLAYERS = int(os.environ.get("KLAYERS", "3"))
KSUB = int(os.environ.get("KSUB", "32"))

N_NODES = 100000
N_EDGES = 1600000
D = 64
H = 16
N_LAYERS = 3
NCORES = 8

B = 12500          # nodes per core
BP = 12544         # padded (98 * 128)
NSW = 98           # subwindows of 128 nodes per core
CH = 2 * BP        # 25088 rows per gather chunk (2 blocks)
NCHUNK = 4
PADIDX = B         # chunk-relative index of a guaranteed-zero table row
NWIN = 25          # windows of 512 nodes (last = 256)
SUB_TILES = KSUB   # gather sub-call size in 128-edge tiles

_BF16 = ml_dtypes.bfloat16


def _preprocess(edge_index):
    """Build per-core padded gather-index and slot arrays (layer-invariant)."""
    src = np.asarray(edge_index[0], dtype=np.int64)
    dst = np.asarray(edge_index[1], dtype=np.int64)
    core = dst // B
    dstL = dst - core * B
    sw = dstL >> 7
    slot = (dstL & 127).astype(np.uint8)
    src_row = (src // B) * BP + (src % B)
    chunk = src_row // CH
    rel = (src_row - chunk * CH).astype(np.int16)

    key = ((core * NCHUNK + chunk) * NSW + sw).astype(np.int64)
    order = np.argsort(key, kind="stable")
    key_s = key[order]
    rel_s = rel[order]
    slot_s = slot[order]
    core_s = core[order]
    chunk_s = chunk[order]

    counts = np.bincount(key, minlength=NCORES * NCHUNK * NSW).reshape(
        NCORES, NCHUNK, NSW
    )
    # tiles per (chunk, sw): max over cores, >= 1
    T = np.maximum(1, -(-counts // 128)).max(axis=0)  # [NCHUNK, NSW]
    cap = T * 128

    grp_start = np.zeros(NCORES * NCHUNK * NSW, dtype=np.int64)
    grp_start[1:] = np.cumsum(counts.ravel())[:-1]
    within = np.arange(len(key_s), dtype=np.int64) - grp_start[key_s]

    base = np.zeros((NCHUNK, NSW), dtype=np.int64)
    base[:, 1:] = np.cumsum(cap, axis=1)[:, :-1]
    dest = base[chunk_s, key_s % NSW] + within

    Lk = cap.sum(axis=1).astype(np.int64)  # padded edge-slots per chunk
    idx_pads = []
    slot_pads = []
    for k in range(NCHUNK):
        ip = np.full((NCORES, Lk[k]), PADIDX, dtype=np.int16)
        sp = np.zeros((NCORES, Lk[k]), dtype=np.uint8)
        m = chunk_s == k
        flat = core_s[m] * Lk[k] + dest[m]
        ip.reshape(-1)[flat] = rel_s[m]
        sp.reshape(-1)[flat] = slot_s[m]
        idx_pads.append(ip)
        slot_pads.append(sp)

    # wrap: idx position i -> [i%16, i//16]; replicate rows 0-15 into 16-31
    idx_wrapped = []
    slot_wrapped = []
    for k in range(NCHUNK):
        iw = np.zeros((NCORES, 128, Lk[k] // 16), dtype=np.int16)
        w = idx_pads[k].reshape(NCORES, -1, 16).transpose(0, 2, 1)
        iw[:, 0:16] = w
        iw[:, 16:32] = w
        idx_wrapped.append(iw)
        slot_wrapped.append(
            slot_pads[k]
            .reshape(NCORES, -1, 128)
            .transpose(0, 2, 1)
            .astype(_BF16)
        )
    idx_all = np.concatenate(idx_wrapped, axis=2)    # [8, 128, sum(Lk)/16]
    slot_all = np.concatenate(slot_wrapped, axis=2)  # [8, 128, sum(Lk)/128]
    return T, Lk, idx_all, slot_all


def _build_program(T, Lk):
    """Build the SPMD bass program (identical for all 8 cores)."""
    import concourse.bass as bass
    import concourse.bacc as bacc
    import concourse.tile as tile
    from concourse import mybir

    dt = mybir.dt
    AF = mybir.ActivationFunctionType
    ALU = mybir.AluOpType

    nc = bacc.Bacc(
        "TRN2", target_bir_lowering=False, debug=False, num_devices=NCORES
    )

    # ---- I/O ----
    x0_d = nc.dram_tensor("x0", [BP, D], dt.float32, kind="ExternalInput")
    idx_cols = int(Lk.sum()) // 16
    slot_cols = int(Lk.sum()) // 128
    idx_d = nc.dram_tensor("idx", [128, idx_cols], dt.int16, kind="ExternalInput")
    slot_d = nc.dram_tensor(
        "slot", [128, slot_cols], dt.bfloat16, kind="ExternalInput"
    )
    wm1_d = nc.dram_tensor("wm1", [N_LAYERS, D, H], dt.bfloat16, kind="ExternalInput")
    bm1_d = nc.dram_tensor("bm1", [N_LAYERS, H, 1], dt.float32, kind="ExternalInput")
    wm2_d = nc.dram_tensor("wm2", [N_LAYERS, H, D], dt.bfloat16, kind="ExternalInput")
    bm2_d = nc.dram_tensor("bm2", [N_LAYERS, D, 1], dt.float32, kind="ExternalInput")
    wu1a_d = nc.dram_tensor("wu1a", [N_LAYERS, D, H], dt.float32, kind="ExternalInput")
    wu1b_d = nc.dram_tensor(
        "wu1b", [N_LAYERS, D, H], dt.bfloat16, kind="ExternalInput"
    )
    bu1_d = nc.dram_tensor("bu1", [N_LAYERS, H, 1], dt.float32, kind="ExternalInput")
    wu2_d = nc.dram_tensor("wu2", [N_LAYERS, H, D], dt.bfloat16, kind="ExternalInput")
    bu2_d = nc.dram_tensor("bu2", [N_LAYERS, D, 1], dt.float32, kind="ExternalInput")
    iota_d = nc.dram_tensor("iota", [128, 128], dt.bfloat16, kind="ExternalInput")
    idenf_d = nc.dram_tensor("idenf", [128, 128], dt.float32, kind="ExternalInput")
    idenb_d = nc.dram_tensor("idenb", [64, 64], dt.bfloat16, kind="ExternalInput")
    out_d = nc.dram_tensor("out", [BP, D], dt.float32, kind="ExternalOutput")

    block_d = nc.dram_tensor("mblock", [BP, 128], dt.bfloat16)
    table_d = nc.dram_tensor(
        "mtable", [NCORES * BP, 128], dt.bfloat16, addr_space="Shared"
    )

    # per-chunk tile schedule
    sw_of_tile = []  # [NCHUNK][tile] -> sw
    tflag = []       # [NCHUNK][tile] -> (is_first, is_last) within (chunk, sw)
    for k in range(NCHUNK):
        sws = []
        fl = []
        for sw in range(NSW):
            t = int(T[k][sw])
            for j in range(t):
                sws.append(sw)
                fl.append((j == 0, j == t - 1))
        sw_of_tile.append(sws)
        tflag.append(fl)
    tiles_k = [len(s) for s in sw_of_tile]
    idx_off16 = np.concatenate([[0], np.cumsum(Lk // 16)]).astype(int)
    slot_offT = np.concatenate([[0], np.cumsum(Lk // 128)]).astype(int)

    from contextlib import ExitStack

    with tile.TileContext(nc) as tc, ExitStack() as ctx:
        P = lambda **kw: ctx.enter_context(tc.tile_pool(**kw))
        cpool = P(name="consts", bufs=1)
        xpool = P(name="xT", bufs=1)
        apool = P(name="aggr", bufs=1)
        slpool = P(name="slot", bufs=1)
        idxpool = P(name="idx", bufs=2)
        gpool = P(name="G", bufs=2)
        spool = P(name="S", bufs=2)
        mpool = P(name="mT", bufs=2)
        hpool = P(name="hid", bufs=2)
        stgpool = P(name="stg", bufs=3)
        xldpool = P(name="xld", bufs=2)
        xopool = P(name="xout", bufs=2)
        ps_seg = P(name="ps_seg", bufs=2, space="PSUM")
        ps_m16 = P(name="ps_m16", bufs=2, space="PSUM")
        ps_m64 = P(name="ps_m64", bufs=2, space="PSUM")
        ps_tr = P(name="ps_tr", bufs=2, space="PSUM")

        # ---- constants ----
        iota_t = cpool.tile([128, 128], dt.bfloat16, tag="iota")
        nc.sync.dma_start(iota_t[:], iota_d[:])
        idenf_t = cpool.tile([128, 128], dt.float32, tag="idenf")
        nc.sync.dma_start(idenf_t[:], idenf_d[:])
        idenb_t = cpool.tile([64, 64], dt.bfloat16, tag="idenb")
        nc.sync.dma_start(idenb_t[:], idenb_d[:])

        def _w(name, dram, shape, dtype):
            ts = []
            for l in range(N_LAYERS):
                t = cpool.tile(shape, dtype, tag=f"{name}{l}")
                nc.sync.dma_start(t[:], dram[l])
                ts.append(t)
            return ts

        wm1_t = _w("wm1", wm1_d, [D, H], dt.bfloat16)
        bm1_t = _w("bm1", bm1_d, [H, 1], dt.float32)
        wm2_t = _w("wm2", wm2_d, [H, D], dt.bfloat16)
        bm2_t = _w("bm2", bm2_d, [D, 1], dt.float32)
        wu1a_t = _w("wu1a", wu1a_d, [D, H], dt.float32)
        wu1b_t = _w("wu1b", wu1b_d, [D, H], dt.bfloat16)
        bu1_t = _w("bu1", bu1_d, [H, 1], dt.float32)
        wu2_t = _w("wu2", wu2_d, [H, D], dt.bfloat16)
        bu2_t = _w("bu2", bu2_d, [D, 1], dt.float32)

        slot_t = slpool.tile([128, slot_cols], dt.bfloat16)
        nc.sync.dma_start(slot_t[:], slot_d[:])

        # ---- x0 -> x^T (bf16) ----
        xA = xpool.tile([D, BP], dt.bfloat16, tag="xA")
        nc.gpsimd.memset(xA[:, B:BP], 0.0)
        for t in range(NSW):
            xld = xldpool.tile([128, D], dt.float32)
            nc.sync.dma_start(xld[:], x0_d[128 * t : 128 * (t + 1), :])
            pt = ps_tr.tile([D, 128], dt.float32)
            nc.tensor.transpose(pt[:], xld[:], idenf_t[:])
            nc.scalar.activation(xA[:, 128 * t : 128 * (t + 1)], pt[:], AF.Copy)

        aggr = apool.tile([D, BP], dt.float32)

        for layer in range(KLAYERS):
            xT = xA
            xN = xA  # in-place update: xT[ch] is dead once the update MLP read it

            # ---- message MLP + transpose + block write ----
            for ch in range(NWIN):
                o = 512 * ch
                wd = min(512, B - o)         # mlp width (real nodes)
                wt = min(512, BP - o)        # transpose width (padded)
                p1 = ps_m16.tile([H, 512], dt.float32)
                nc.tensor.matmul(
                    p1[:, :wd], lhsT=wm1_t[layer][:], rhs=xT[:, o : o + wd],
                    start=True, stop=True,
                )
                hid = hpool.tile([H, 512], dt.bfloat16, tag="hid")
                nc.scalar.activation(
                    hid[:, :wd], p1[:, :wd], AF.Relu, bias=bm1_t[layer][:]
                )
                p2 = ps_m64.tile([D, 512], dt.float32)
                nc.tensor.matmul(
                    p2[:, :wd], lhsT=wm2_t[layer][:], rhs=hid[:, :wd],
                    start=True, stop=True,
                )
                mt = mpool.tile([D, 512], dt.bfloat16)
                nc.scalar.activation(
                    mt[:, :wd], p2[:, :wd], AF.Relu, bias=bm2_t[layer][:]
                )
                if wd < wt:
                    nc.gpsimd.memset(mt[:, wd:wt], 0.0)
                for st in range(-(-wt // 128)):
                    so = 128 * st
                    pt = ps_tr.tile([128, D], dt.bfloat16)
                    nc.tensor.transpose(
                        pt[:], mt[:, so : so + 128], idenb_t[:]
                    )
                    stg = stgpool.tile([128, D], dt.bfloat16, tag="stgb")
                    nc.scalar.activation(stg[:], pt[:], AF.Copy)
                    nc.sync.dma_start(
                        block_d[o + so : o + so + 128, 0:D], stg[:]
                    )

            # ---- AllGather message table ----
            if KPHASE >= 2:
                nc.gpsimd.collective_compute(
                    "AllGather",
                    mybir.AluOpType.bypass,
                    replica_groups=[list(range(NCORES))],
                    ins=[block_d[:].opt()],
                    outs=[table_d[:].opt()],
                )

            # ---- gather + segment matmul passes ----
            for k in range(NCHUNK if KPHASE >= 3 else 0):
                ncols = int(Lk[k]) // 16
                idx_t = idxpool.tile([128, ncols], dt.int16)
                nc.sync.dma_start(
                    idx_t[:], idx_d[:, idx_off16[k] : idx_off16[k] + ncols]
                )
                table_k = table_d[CH * k : CH * (k + 1), :]

                ntiles = tiles_k[k]
                nsub = -(-ntiles // SUB_TILES)
                cur_win = -1
                pw = None
                tb = 0  # tile base within this chunk
                for si in range(nsub):
                    nt = min(SUB_TILES, ntiles - tb)
                    g = gpool.tile([128, SUB_TILES, 128], dt.bfloat16)
                    nidx = nt * 128
                    nc.gpsimd.dma_gather(
                        g[:, :nt, :],
                        table_k,
                        idx_t[:, 8 * tb : 8 * (tb + nt)],
                        nidx,
                        nidx,
                        128,
                        single_packet=False,
                    )
                    sb = spool.tile([128, SUB_TILES, 128], dt.bfloat16)
                    io_b = iota_t[:].unsqueeze(1).broadcast_to([128, nt, 128])
                    sl_b = (
                        slot_t[:, slot_offT[k] + tb : slot_offT[k] + tb + nt]
                        .unsqueeze(2)
                        .broadcast_to([128, nt, 128])
                    )
                    nc.vector.tensor_tensor(sb[:, :nt, :], io_b, sl_b, ALU.is_equal)

                    for j in range(nt if KPHASE >= 4 else 0):
                        ti = tb + j
                        sw = sw_of_tile[k][ti]
                        first, last = tflag[k][ti]
                        win = sw // 4
                        if win != cur_win:
                            if pw is not None:
                                _evac(nc, ALU, aggr, pw, cur_win, k)
                            pw = ps_seg.tile([D, 512], dt.float32)
                            cur_win = win
                        nc.tensor.matmul(
                            pw[:, 128 * (sw % 4) : 128 * (sw % 4) + 128],
                            lhsT=g[:, j, 0:D],
                            rhs=sb[:, j, :],
                            start=first,
                            stop=last,
                        )
                    tb += nt
                if pw is not None:
                    _evac(nc, ALU, aggr, pw, cur_win, k)

            # ---- update MLP ----
            if KPHASE < 4:
                nc.vector.tensor_copy(aggr[:], xT[:])  # placeholder so aggr defined
            for ch in range(NWIN if KPHASE >= 5 else 0):
                o = 512 * ch
                wd = min(512, B - o)
                p1 = ps_m16.tile([H, 512], dt.float32)
                nc.tensor.matmul(
                    p1[:, :wd], lhsT=wu1a_t[layer][:], rhs=aggr[:, o : o + wd],
                    start=True, stop=False, skip_group_check=True,
                )
                nc.tensor.matmul(
                    p1[:, :wd], lhsT=wu1b_t[layer][:], rhs=xT[:, o : o + wd],
                    start=False, stop=True, skip_group_check=True,
                )
                hid = hpool.tile([H, 512], dt.bfloat16, tag="hid")
                nc.scalar.activation(
                    hid[:, :wd], p1[:, :wd], AF.Relu, bias=bu1_t[layer][:]
                )
                p2 = ps_m64.tile([D, 512], dt.float32)
                nc.tensor.matmul(
                    p2[:, :wd], lhsT=wu2_t[layer][:], rhs=hid[:, :wd],
                    start=True, stop=True,
                )
                if layer < N_LAYERS - 1:
                    nc.scalar.activation(
                        xN[:, o : o + wd], p2[:, :wd], AF.Relu,
                        bias=bu2_t[layer][:],
                    )
                else:
                    xo = xopool.tile([D, 512], dt.float32)
                    nc.scalar.activation(
                        xo[:, :wd], p2[:, :wd], AF.Relu, bias=bu2_t[layer][:]
                    )
                    for st in range(-(-wd // 128)):
                        so = 128 * st
                        tw = min(128, wd - so)
                        pt = ps_tr.tile([128, D], dt.float32)
                        nc.tensor.transpose(
                            pt[:tw, :], xo[:, so : so + tw], idenf_t[0:D, 0:D]
                        )
                        stg = stgpool.tile([128, D], dt.float32, tag="stgf")
                        nc.scalar.activation(stg[:tw, :], pt[:tw, :], AF.Copy)
                        nc.sync.dma_start(
                            out_d[o + so : o + so + tw, :], stg[:tw, :]
                        )
        if KPHASE < 5:
            zo = stgpool.tile([128, D], dt.float32, tag="stgf")
            nc.gpsimd.memset(zo[:], 0.0)
            for ch0 in range(NSW):
                nc.sync.dma_start(out_d[128 * ch0 : 128 * (ch0 + 1), :], zo[:])
    nc.finalize()
    return nc


def _evac(nc, ALU, aggr, pw, win, k):
    o = 512 * win
    wd = min(512, BP - o)
    if k == 0:
        nc.vector.tensor_copy(aggr[:, o : o + wd], pw[:, :wd])
    else:
        nc.vector.tensor_tensor(
            aggr[:, o : o + wd], pw[:, :wd], aggr[:, o : o + wd], ALU.add
        )


_CACHE = {}


def _make_in_maps(inputs, idx_all, slot_all):
    x = np.asarray(inputs["x"], dtype=np.float32)
    Wm1, bm1 = inputs["Wm1"], inputs["bm1"]
    Wm2, bm2 = inputs["Wm2"], inputs["bm2"]
    Wu1, bu1 = inputs["Wu1"], inputs["bu1"]
    Wu2, bu2 = inputs["Wu2"], inputs["bu2"]

    xpad = np.zeros((NCORES, BP, D), dtype=np.float32)
    xpad[:, :B] = x.reshape(NCORES, B, D)

    iota = np.broadcast_to(np.arange(128, dtype=np.float32), (128, 128)).astype(_BF16)
    idenf = np.eye(128, dtype=np.float32)
    idenb = np.eye(64, dtype=np.float32).astype(_BF16)

    com = {
        "wm1": np.ascontiguousarray(np.asarray(Wm1, np.float32)).astype(_BF16),
        "bm1": np.asarray(bm1, np.float32).reshape(N_LAYERS, H, 1),
        "wm2": np.ascontiguousarray(np.asarray(Wm2, np.float32)).astype(_BF16),
        "bm2": np.asarray(bm2, np.float32).reshape(N_LAYERS, D, 1),
        "wu1a": np.ascontiguousarray(np.asarray(Wu1, np.float32)[:, :D, :]),
        "wu1b": np.ascontiguousarray(np.asarray(Wu1, np.float32)[:, D:, :]).astype(
            _BF16
        ),
        "bu1": np.asarray(bu1, np.float32).reshape(N_LAYERS, H, 1),
        "wu2": np.ascontiguousarray(np.asarray(Wu2, np.float32)).astype(_BF16),
        "bu2": np.asarray(bu2, np.float32).reshape(N_LAYERS, D, 1),
        "iota": iota,
        "idenf": idenf,
        "idenb": idenb,
    }
    in_maps = []
    for c in range(NCORES):
        m = dict(com)
        m["x0"] = xpad[c]
        m["idx"] = np.ascontiguousarray(idx_all[c])
        m["slot"] = np.ascontiguousarray(slot_all[c])
        in_maps.append(m)
    return in_maps


def kernel(x, edge_index, Wm1, bm1, Wm2, bm2, Wu1, bu1, Wu2, bu2):
    from concourse.bass_utils import run_bass_kernel_spmd

    ei_key = hash(np.asarray(edge_index)[:, ::97].tobytes())
    if ei_key not in _CACHE:
        T, Lk, idx_all, slot_all = _preprocess(edge_index)
        nc = _build_program(T, Lk)
        _CACHE[ei_key] = (nc, idx_all, slot_all)
    nc, idx_all, slot_all = _CACHE[ei_key]

    in_maps = _make_in_maps(
        dict(
            x=x, edge_index=edge_index, Wm1=Wm1, bm1=bm1, Wm2=Wm2, bm2=bm2,
            Wu1=Wu1, bu1=bu1, Wu2=Wu2, bu2=bu2,
        ),
        idx_all,
        slot_all,
    )
    res = run_bass_kernel_spmd(nc, in_maps, core_ids=list(range(NCORES)))
    out = np.concatenate([res.results[c]["out"][:B] for c in range(NCORES)], axis=0)
    return out


if __name__ == "__main__":
    import reference

    inputs = reference.setup_inputs()
    inputs = {k: np.asarray(v) for k, v in inputs.items()}
    got = kernel(**inputs)
    exp = np.asarray(reference.reference(**{k: v for k, v in inputs.items()}))
    err = np.abs(got - exp)
    rel = np.linalg.norm(got - exp) / np.linalg.norm(exp)
    print("max abs err:", err.max(), "rel:", rel)



# revision 13
# speedup vs baseline: 1.6040x; 1.1718x over previous
"""GNN message-passing kernel for 8 Trainium2 NeuronCores.

Math: 3 layers of
    m   = relu(relu(x[src] @ Wm1 + bm1) @ Wm2 + bm2)      # message MLP
    aggr= segment_sum(m, dst)                              # scatter-add
    x   = relu(relu([aggr, x] @ Wu1 + bu1) @ Wu2 + bu2)    # update MLP

Key algorithmic point: the message MLP depends only on x[src], so it is
computed per-NODE (100k rows) not per-EDGE (1.6M rows); the edge work
reduces to gather + segment-sum, done as:
  - per-layer AllGather of the per-node message table (bf16, padded rows)
  - dma_gather of 256B rows (4 passes over 25088-row chunks: int16 index)
  - segment-sum via PE matmul against a one-hot slot matrix built on DVE
"""

import sys

sys.path.insert(0, "/opt/trn_rl_repo")

import os
import numpy as np
import ml_dtypes

KPHASE = int(os.environ.get("KPHASE", "5"))
KLAYERS = int(os.environ.get("KLAYERS", "3"))
KSUB = int(os.environ.get("KSUB", "32"))

N_NODES = 100000
N_EDGES = 1600000
D = 64
H = 16
N_LAYERS = 3
NCORES = 8

B = 12500          # nodes per core
BP = 12544         # padded (98 * 128)
NSW = 98           # subwindows of 128 nodes per core
CH = 2 * BP        # 25088 rows per gather chunk (2 blocks)
NCHUNK = 4
PADIDX = B         # chunk-relative index of a guaranteed-zero table row
NWIN = 25          # windows of 512 nodes (last = 256)
SUB_TILES = KSUB   # gather sub-call size in 128-edge tiles
SB_B = 8           # one-hot build batch (tiles per is_equal)

_BF16 = ml_dtypes.bfloat16


def _preprocess(edge_index):
    """Build per-core padded gather-index and slot arrays (layer-invariant)."""
    src = np.asarray(edge_index[0], dtype=np.int64)
    dst = np.asarray(edge_index[1], dtype=np.int64)
    core = dst // B
    dstL = dst - core * B
    win = dstL >> 9
    slot = (dstL & 511).astype(np.uint16)
    src_row = (src // B) * BP + (src % B)
    chunk = src_row // CH
    rel = (src_row - chunk * CH).astype(np.int16)

    key = ((core * NCHUNK + chunk) * NWIN + win).astype(np.int64)
    order = np.argsort(key, kind="stable")
    key_s = key[order]
    rel_s = rel[order]
    slot_s = slot[order]
    core_s = core[order]
    chunk_s = chunk[order]

    counts = np.bincount(key, minlength=NCORES * NCHUNK * NWIN).reshape(
        NCORES, NCHUNK, NWIN
    )
    # tiles per (chunk, win): max over cores, >= 1
    T = np.maximum(1, -(-counts // 128)).max(axis=0)  # [NCHUNK, NWIN]
    cap = T * 128

    grp_start = np.zeros(NCORES * NCHUNK * NWIN, dtype=np.int64)
    grp_start[1:] = np.cumsum(counts.ravel())[:-1]
    within = np.arange(len(key_s), dtype=np.int64) - grp_start[key_s]

    base = np.zeros((NCHUNK, NWIN), dtype=np.int64)
    base[:, 1:] = np.cumsum(cap, axis=1)[:, :-1]
    dest = base[chunk_s, key_s % NWIN] + within

    Lk = cap.sum(axis=1).astype(np.int64)  # padded edge-slots per chunk
    idx_pads = []
    slot_pads = []
    for k in range(NCHUNK):
        ip = np.full((NCORES, Lk[k]), PADIDX, dtype=np.int16)
        sp = np.zeros((NCORES, Lk[k]), dtype=np.uint16)
        m = chunk_s == k
        flat = core_s[m] * Lk[k] + dest[m]
        ip.reshape(-1)[flat] = rel_s[m]
        sp.reshape(-1)[flat] = slot_s[m]
        idx_pads.append(ip)
        slot_pads.append(sp)

    # wrap: idx position i -> [i%16, i//16]; replicate rows 0-15 into 16-31
    idx_wrapped = []
    slot_wrapped = []
    for k in range(NCHUNK):
        iw = np.zeros((NCORES, 128, Lk[k] // 16), dtype=np.int16)
        w = idx_pads[k].reshape(NCORES, -1, 16).transpose(0, 2, 1)
        iw[:, 0:16] = w
        iw[:, 16:32] = w
        idx_wrapped.append(iw)
        slot_wrapped.append(
            slot_pads[k]
            .reshape(NCORES, -1, 128)
            .transpose(0, 2, 1)
            .astype(np.float16)
        )
    idx_all = np.concatenate(idx_wrapped, axis=2)    # [8, 128, sum(Lk)/16]
    slot_all = np.concatenate(slot_wrapped, axis=2)  # [8, 128, sum(Lk)/128]
    return T, Lk, idx_all, slot_all


def _build_program(T, Lk):
    """Build the SPMD bass program (identical for all 8 cores)."""
    import concourse.bass as bass
    import concourse.bacc as bacc
    import concourse.tile as tile
    from concourse import mybir

    dt = mybir.dt
    AF = mybir.ActivationFunctionType
    ALU = mybir.AluOpType

    nc = bacc.Bacc(
        "TRN2", target_bir_lowering=False, debug=False, num_devices=NCORES
    )

    # ---- I/O ----
    x0_d = nc.dram_tensor("x0", [BP, D], dt.float32, kind="ExternalInput")
    idx_cols = int(Lk.sum()) // 16
    slot_cols = int(Lk.sum()) // 128
    idx_d = nc.dram_tensor("idx", [128, idx_cols], dt.int16, kind="ExternalInput")
    slot_d = nc.dram_tensor(
        "slot", [128, slot_cols], dt.float16, kind="ExternalInput"
    )
    wm1_d = nc.dram_tensor("wm1", [N_LAYERS, D, H], dt.bfloat16, kind="ExternalInput")
    bm1_d = nc.dram_tensor("bm1", [N_LAYERS, H, 1], dt.float32, kind="ExternalInput")
    wm2_d = nc.dram_tensor("wm2", [N_LAYERS, H, D], dt.bfloat16, kind="ExternalInput")
    bm2_d = nc.dram_tensor("bm2", [N_LAYERS, D, 1], dt.float32, kind="ExternalInput")
    wu1a_d = nc.dram_tensor("wu1a", [N_LAYERS, D, H], dt.float32, kind="ExternalInput")
    wu1b_d = nc.dram_tensor(
        "wu1b", [N_LAYERS, D, H], dt.bfloat16, kind="ExternalInput"
    )
    bu1_d = nc.dram_tensor("bu1", [N_LAYERS, H, 1], dt.float32, kind="ExternalInput")
    wu2_d = nc.dram_tensor("wu2", [N_LAYERS, H, D], dt.bfloat16, kind="ExternalInput")
    bu2_d = nc.dram_tensor("bu2", [N_LAYERS, D, 1], dt.float32, kind="ExternalInput")
    iota_d = nc.dram_tensor("iota", [128, 512], dt.float16, kind="ExternalInput")
    idenf_d = nc.dram_tensor("idenf", [128, 128], dt.float32, kind="ExternalInput")
    idenb_d = nc.dram_tensor("idenb", [64, 64], dt.float16, kind="ExternalInput")
    out_d = nc.dram_tensor("out", [BP, D], dt.float32, kind="ExternalOutput")

    block_d = nc.dram_tensor("mblock", [BP, 128], dt.float16)
    table_d = nc.dram_tensor(
        "mtable", [NCORES * BP, 128], dt.float16, addr_space="Shared"
    )

    # per-chunk tile schedule
    win_of_tile = []  # [NCHUNK][tile] -> 512-slot window
    tflag = []        # [NCHUNK][tile] -> (is_first, is_last) within (chunk, win)
    for k in range(NCHUNK):
        ws = []
        fl = []
        for w in range(NWIN):
            t = int(T[k][w])
            for j in range(t):
                ws.append(w)
                fl.append((j == 0, j == t - 1))
        win_of_tile.append(ws)
        tflag.append(fl)
    tiles_k = [len(s) for s in win_of_tile]
    idx_off16 = np.concatenate([[0], np.cumsum(Lk // 16)]).astype(int)
    slot_offT = np.concatenate([[0], np.cumsum(Lk // 128)]).astype(int)

    from contextlib import ExitStack

    with tile.TileContext(nc) as tc, ExitStack() as ctx:
        P = lambda **kw: ctx.enter_context(tc.tile_pool(**kw))
        cpool = P(name="consts", bufs=1)
        xpool = P(name="xT", bufs=1)
        apool = P(name="aggr", bufs=1)
        slpool = P(name="slot", bufs=1)
        idxpool = P(name="idx", bufs=2)
        gpool = P(name="G", bufs=2)
        spool = P(name="S", bufs=2)
        mpool = P(name="mT", bufs=2)
        hpool = P(name="hid", bufs=2)
        stgpool = P(name="stg", bufs=3)
        xldpool = P(name="xld", bufs=2)
        xopool = P(name="xout", bufs=2)
        ps_seg = P(name="ps_seg", bufs=2, space="PSUM")
        ps_m16 = P(name="ps_m16", bufs=2, space="PSUM")
        ps_m64 = P(name="ps_m64", bufs=2, space="PSUM")
        ps_tr = P(name="ps_tr", bufs=2, space="PSUM")

        # ---- constants ----
        iota_t = cpool.tile([128, 512], dt.float16, tag="iota")
        nc.sync.dma_start(iota_t[:], iota_d[:])
        idenf_t = cpool.tile([128, 128], dt.float32, tag="idenf")
        nc.sync.dma_start(idenf_t[:], idenf_d[:])
        idenb_t = cpool.tile([64, 64], dt.float16, tag="idenb")
        nc.sync.dma_start(idenb_t[:], idenb_d[:])

        def _w(name, dram, shape, dtype):
            ts = []
            for l in range(N_LAYERS):
                t = cpool.tile(shape, dtype, tag=f"{name}{l}")
                nc.sync.dma_start(t[:], dram[l])
                ts.append(t)
            return ts

        wm1_t = _w("wm1", wm1_d, [D, H], dt.bfloat16)
        bm1_t = _w("bm1", bm1_d, [H, 1], dt.float32)
        wm2_t = _w("wm2", wm2_d, [H, D], dt.bfloat16)
        bm2_t = _w("bm2", bm2_d, [D, 1], dt.float32)
        wu1a_t = _w("wu1a", wu1a_d, [D, H], dt.float32)
        wu1b_t = _w("wu1b", wu1b_d, [D, H], dt.bfloat16)
        bu1_t = _w("bu1", bu1_d, [H, 1], dt.float32)
        wu2_t = _w("wu2", wu2_d, [H, D], dt.bfloat16)
        bu2_t = _w("bu2", bu2_d, [D, 1], dt.float32)

        slot_t = slpool.tile([128, slot_cols], dt.float16)
        nc.sync.dma_start(slot_t[:], slot_d[:])

        # ---- x0 -> x^T (bf16) ----
        xA = xpool.tile([D, BP], dt.bfloat16, tag="xA")
        nc.gpsimd.memset(xA[:, B:BP], 0.0)
        for t in range(NSW):
            xld = xldpool.tile([128, D], dt.float32)
            nc.sync.dma_start(xld[:], x0_d[128 * t : 128 * (t + 1), :])
            pt = ps_tr.tile([D, 128], dt.float32)
            nc.tensor.transpose(pt[:], xld[:], idenf_t[:])
            nc.scalar.activation(xA[:, 128 * t : 128 * (t + 1)], pt[:], AF.Copy)

        aggr = apool.tile([D, BP], dt.float32)

        for layer in range(KLAYERS):
            xT = xA
            xN = xA  # in-place update: xT[ch] is dead once the update MLP read it

            # ---- message MLP + transpose + block write ----
            for ch in range(NWIN):
                o = 512 * ch
                wd = min(512, B - o)         # mlp width (real nodes)
                wt = min(512, BP - o)        # transpose width (padded)
                p1 = ps_m16.tile([H, 512], dt.float32)
                nc.tensor.matmul(
                    p1[:, :wd], lhsT=wm1_t[layer][:], rhs=xT[:, o : o + wd],
                    start=True, stop=True,
                )
                hid = hpool.tile([H, 512], dt.bfloat16, tag="hid")
                nc.scalar.activation(
                    hid[:, :wd], p1[:, :wd], AF.Relu, bias=bm1_t[layer][:]
                )
                p2 = ps_m64.tile([D, 512], dt.float32)
                nc.tensor.matmul(
                    p2[:, :wd], lhsT=wm2_t[layer][:], rhs=hid[:, :wd],
                    start=True, stop=True,
                )
                mt = mpool.tile([D, 512], dt.float16)
                nc.scalar.activation(
                    mt[:, :wd], p2[:, :wd], AF.Relu, bias=bm2_t[layer][:]
                )
                if wd < wt:
                    nc.gpsimd.memset(mt[:, wd:wt], 0.0)
                for st in range(-(-wt // 128)):
                    so = 128 * st
                    pt = ps_tr.tile([128, D], dt.float16)
                    nc.tensor.transpose(
                        pt[:], mt[:, so : so + 128], idenb_t[:]
                    )
                    stg = stgpool.tile([128, D], dt.float16, tag="stgb")
                    nc.scalar.activation(stg[:], pt[:], AF.Copy)
                    nc.sync.dma_start(
                        block_d[o + so : o + so + 128, 0:D], stg[:]
                    )

            # ---- AllGather message table ----
            if KPHASE >= 2:
                nc.gpsimd.collective_compute(
                    "AllGather",
                    mybir.AluOpType.bypass,
                    replica_groups=[list(range(NCORES))],
                    ins=[block_d[:].opt()],
                    outs=[table_d[:].opt()],
                )

            # ---- gather + segment matmul passes ----
            for k in range(NCHUNK if KPHASE >= 3 else 0):
                ncols = int(Lk[k]) // 16
                idx_t = idxpool.tile([128, ncols], dt.int16)
                nc.sync.dma_start(
                    idx_t[:], idx_d[:, idx_off16[k] : idx_off16[k] + ncols]
                )
                table_k = table_d[CH * k : CH * (k + 1), :]

                ntiles = tiles_k[k]
                nsub = -(-ntiles // SUB_TILES)
                cur_win = -1
                pw = None
                tb = 0  # tile base within this chunk
                for si in range(nsub):
                    nt = min(SUB_TILES, ntiles - tb)
                    g = gpool.tile([128, SUB_TILES, 128], dt.float16)
                    nidx = nt * 128
                    nc.gpsimd.dma_gather(
                        g[:, :nt, :],
                        table_k,
                        idx_t[:, 8 * tb : 8 * (tb + nt)],
                        nidx,
                        nidx,
                        128,
                        single_packet=False,
                    )
                    for bb in range(0, nt if KPHASE >= 4 else 0, SB_B):
                        nb = min(SB_B, nt - bb)
                        sb = spool.tile([128, SB_B, 512], dt.float16)
                        io_b = iota_t[:].unsqueeze(1).broadcast_to([128, nb, 512])
                        sl_b = (
                            slot_t[
                                :,
                                slot_offT[k] + tb + bb : slot_offT[k] + tb + bb + nb,
                            ]
                            .unsqueeze(2)
                            .broadcast_to([128, nb, 512])
                        )
                        nc.vector.tensor_tensor(
                            sb[:, :nb, :], io_b, sl_b, ALU.is_equal
                        )
                        for j2 in range(nb):
                            ti = tb + bb + j2
                            first, last = tflag[k][ti]
                            win = win_of_tile[k][ti]
                            if win != cur_win:
                                if pw is not None:
                                    _evac(nc, ALU, aggr, pw, cur_win, k)
                                pw = ps_seg.tile([D, 512], dt.float32)
                                cur_win = win
                            nc.tensor.matmul(
                                pw[:, :],
                                lhsT=g[:, bb + j2, 0:D],
                                rhs=sb[:, j2, :],
                                start=first,
                                stop=last,
                            )
                    tb += nt
                if pw is not None:
                    _evac(nc, ALU, aggr, pw, cur_win, k)

            # ---- update MLP ----
            if KPHASE < 4:
                nc.vector.tensor_copy(aggr[:], xT[:])  # placeholder so aggr defined
            for ch in range(NWIN if KPHASE >= 5 else 0):
                o = 512 * ch
                wd = min(512, B - o)
                p1 = ps_m16.tile([H, 512], dt.float32)
                nc.tensor.matmul(
                    p1[:, :wd], lhsT=wu1a_t[layer][:], rhs=aggr[:, o : o + wd],
                    start=True, stop=False, skip_group_check=True,
                )
                nc.tensor.matmul(
                    p1[:, :wd], lhsT=wu1b_t[layer][:], rhs=xT[:, o : o + wd],
                    start=False, stop=True, skip_group_check=True,
                )
                hid = hpool.tile([H, 512], dt.bfloat16, tag="hid")
                nc.scalar.activation(
                    hid[:, :wd], p1[:, :wd], AF.Relu, bias=bu1_t[layer][:]
                )
                p2 = ps_m64.tile([D, 512], dt.float32)
                nc.tensor.matmul(
                    p2[:, :wd], lhsT=wu2_t[layer][:], rhs=hid[:, :wd],
                    start=True, stop=True,
                )
                if layer < N_LAYERS - 1:
                    nc.scalar.activation(
                        xN[:, o : o + wd], p2[:, :wd], AF.Relu,
                        bias=bu2_t[layer][:],
                    )
                else:
                    xo = xopool.tile([D, 512], dt.float32)
                    nc.scalar.activation(
                        xo[:, :wd], p2[:, :wd], AF.Relu, bias=bu2_t[layer][:]
                    )
                    for st in range(-(-wd // 128)):
                        so = 128 * st
                        tw = min(128, wd - so)
                        pt = ps_tr.tile([128, D], dt.float32)
                        nc.tensor.transpose(
                            pt[:tw, :], xo[:, so : so + tw], idenf_t[0:D, 0:D]
                        )
                        stg = stgpool.tile([128, D], dt.float32, tag="stgf")
                        nc.scalar.activation(stg[:tw, :], pt[:tw, :], AF.Copy)
                        nc.sync.dma_start(
                            out_d[o + so : o + so + tw, :], stg[:tw, :]
                        )
        if KPHASE < 5:
            zo = stgpool.tile([128, D], dt.float32, tag="stgf")
            nc.gpsimd.memset(zo[:], 0.0)
            for ch0 in range(NSW):
                nc.sync.dma_start(out_d[128 * ch0 : 128 * (ch0 + 1), :], zo[:])
    nc.finalize()
    return nc


def _evac(nc, ALU, aggr, pw, win, k):
    o = 512 * win
    wd = min(512, BP - o)
    if k == 0:
        nc.vector.tensor_copy(aggr[:, o : o + wd], pw[:, :wd])
    else:
        nc.vector.tensor_tensor(
            aggr[:, o : o + wd], pw[:, :wd], aggr[:, o : o + wd], ALU.add
        )


_CACHE = {}


def _make_in_maps(inputs, idx_all, slot_all):
    x = np.asarray(inputs["x"], dtype=np.float32)
    Wm1, bm1 = inputs["Wm1"], inputs["bm1"]
    Wm2, bm2 = inputs["Wm2"], inputs["bm2"]
    Wu1, bu1 = inputs["Wu1"], inputs["bu1"]
    Wu2, bu2 = inputs["Wu2"], inputs["bu2"]

    xpad = np.zeros((NCORES, BP, D), dtype=np.float32)
    xpad[:, :B] = x.reshape(NCORES, B, D)

    iota = np.broadcast_to(np.arange(512, dtype=np.float32), (128, 512)).astype(
        np.float16
    )
    idenf = np.eye(128, dtype=np.float32)
    idenb = np.eye(64, dtype=np.float32).astype(np.float16)

    com = {
        "wm1": np.ascontiguousarray(np.asarray(Wm1, np.float32)).astype(_BF16),
        "bm1": np.asarray(bm1, np.float32).reshape(N_LAYERS, H, 1),
        "wm2": np.ascontiguousarray(np.asarray(Wm2, np.float32)).astype(_BF16),
        "bm2": np.asarray(bm2, np.float32).reshape(N_LAYERS, D, 1),
        "wu1a": np.ascontiguousarray(np.asarray(Wu1, np.float32)[:, :D, :]),
        "wu1b": np.ascontiguousarray(np.asarray(Wu1, np.float32)[:, D:, :]).astype(
            _BF16
        ),
        "bu1": np.asarray(bu1, np.float32).reshape(N_LAYERS, H, 1),
        "wu2": np.ascontiguousarray(np.asarray(Wu2, np.float32)).astype(_BF16),
        "bu2": np.asarray(bu2, np.float32).reshape(N_LAYERS, D, 1),
        "iota": iota,
        "idenf": idenf,
        "idenb": idenb,
    }
    in_maps = []
    for c in range(NCORES):
        m = dict(com)
        m["x0"] = xpad[c]
        m["idx"] = np.ascontiguousarray(idx_all[c])
        m["slot"] = np.ascontiguousarray(slot_all[c])
        in_maps.append(m)
    return in_maps


def kernel(x, edge_index, Wm1, bm1, Wm2, bm2, Wu1, bu1, Wu2, bu2):
    from concourse.bass_utils import run_bass_kernel_spmd

    ei_key = hash(np.asarray(edge_index)[:, ::97].tobytes())
    if ei_key not in _CACHE:
        T, Lk, idx_all, slot_all = _preprocess(edge_index)
        nc = _build_program(T, Lk)
        _CACHE[ei_key] = (nc, idx_all, slot_all)
    nc, idx_all, slot_all = _CACHE[ei_key]

    in_maps = _make_in_maps(
        dict(
            x=x, edge_index=edge_index, Wm1=Wm1, bm1=bm1, Wm2=Wm2, bm2=bm2,
            Wu1=Wu1, bu1=bu1, Wu2=Wu2, bu2=bu2,
        ),
        idx_all,
        slot_all,
    )
    res = run_bass_kernel_spmd(nc, in_maps, core_ids=list(range(NCORES)))
    out = np.concatenate([res.results[c]["out"][:B] for c in range(NCORES)], axis=0)
    return out


if __name__ == "__main__":
    import reference

    inputs = reference.setup_inputs()
    inputs = {k: np.asarray(v) for k, v in inputs.items()}
    got = kernel(**inputs)
    exp = np.asarray(reference.reference(**{k: v for k, v in inputs.items()}))
    err = np.abs(got - exp)
    rel = np.linalg.norm(got - exp) / np.linalg.norm(exp)
    print("max abs err:", err.max(), "rel:", rel)

